# revision 13
# baseline (speedup 1.0000x reference)
"""EntropyGate fused kernel for 8 Trainium2 NeuronCores (axon-tunneled).

Problem (hardcoded shapes): B=4, S=4096, D=2048, window=8.
  H = entropy of softmax over sliding causal window (8) of token L2 norms of x
  gate_in = [y_ssm | y_attn | H]  (B,S,2D+1)
  h = silu(gate_in @ W1 + b1); g = sigmoid(h @ W2 + b2)
  out = g*y_ssm + (1-g)*y_attn

Sharding: flatten tokens (B*S = 16384) -> 8 shards of 2048 tokens (each shard
lies within one sequence). Gate MLP weights replicated on-device via a
device-to-device broadcast (the axon host link is ~60MB/s; D2D is ~4x faster
and runs off the host wire).

Wire-traffic design (the axon tunnel dominates wall time; on-device compute
is ~1ms/core):
  - y_ssm/y_attn ship as int8 [2D, TOK] with per-feature scales folded into
    W1 host-side.
  - W1/W2 ship as int8 with per-output-column scales; the dequant scale is
    applied by the silu/sigmoid epilogue (activation computes
    func(in*scale + bias) and the psum partition dim IS the output channel).
    Combined quantization error lands ~5e-3 on the output, well inside the
    2e-2 gate.
  - token norms m = ||x_t|| ship as a tiny f32 vector per core instead of x
    itself (67MB); the windowed softmax entropy math stays on-device.
  - weights cross the wire once (to core 0) and fan out device-to-device.
  - the kernel returns the gate g quantized to uint8 (DVE converts f32->u8
    with round-to-nearest); the final elementwise mix out = ya + g*(ys-ya)
    runs on host in f32 from the original inputs.
  - donated output zero-buffers are created on-device; output shards are
    fetched in core order so early gates stream back while later cores'
    inputs are still going out (the tunnel is full-duplex).
"""

import numpy as np

P = 128
D = 2048
TOK = 2048        # tokens per core
HALF = 1024       # token half processed per pass
NT = 512          # psum n-tile (fp32 PSUM bank limit)
MT = 16           # d_out tiles of 128
KC = 32           # 128-row feature chunks of [qs; qa]
K2 = 16           # contraction chunks for mm2
WIN = 8
EXT = TOK + WIN - 1   # 2055
N_CORES = 8
B, S = 4, 4096
GSCALE = 255.0        # g is quantized as rne(g*GSCALE) into uint8

# aux tensor layout (f32), per core
OFF_M = 0             # m_ext: EXT norms (7-halo + TOK), zero-padded to 2176
OFF_S1 = 2176         # scale1[m] for mm1 epilogue (2048)
OFF_S2 = 4224         # scale2[e] for mm2 epilogue (2048)
OFF_B1 = 6272         # b1 (2048)
OFF_B2 = 8320         # b2 (2048)
OFF_WH = 10368        # W1 H-row / scale1 (2048)
AUXN = 12416

_CACHE = {}


def _build_nc():
    import concourse.bass as bass
    import concourse.tile as tile
    import concourse.mybir as mybir
    from concourse import bacc
    from contextlib import ExitStack

    f32 = mybir.dt.float32
    bf16 = mybir.dt.bfloat16
    i8 = mybir.dt.int8
    u8 = mybir.dt.uint8
    AF = mybir.ActivationFunctionType
    AX = mybir.AxisListType
    ALU = mybir.AluOpType

    nc = bacc.Bacc("TRN2", target_bir_lowering=False, debug=False, num_devices=1)

    q = nc.dram_tensor("q", [2 * D, TOK], i8, kind="ExternalInput")
    aux = nc.dram_tensor("aux", [AUXN], f32, kind="ExternalInput")
    wq1 = nc.dram_tensor("wq1", [2 * D, D], i8, kind="ExternalInput")
    wq2 = nc.dram_tensor("wq2", [D, D], i8, kind="ExternalInput")
    gout = nc.dram_tensor("gout", [D, TOK], u8, kind="ExternalOutput")
    # per-token-half entropy scratch (separate tensors keep the two entropy
    # pipelines independent in the dependency tracker)
    h_scr = [nc.dram_tensor(f"h_scr{i}", [HALF], bf16, kind="Internal")
             for i in range(2)]

    with tile.TileContext(nc) as tc:
        with ExitStack() as ctx:
            smol = ctx.enter_context(tc.tile_pool(name="smol", bufs=2))
            const = ctx.enter_context(tc.tile_pool(name="const", bufs=1))
            gate = ctx.enter_context(tc.tile_pool(name="gate", bufs=34))
            q8p = ctx.enter_context(tc.tile_pool(name="q8p", bufs=4))
            w18p = ctx.enter_context(tc.tile_pool(name="w18p", bufs=4))
            w28p = ctx.enter_context(tc.tile_pool(name="w28p", bufs=4))
            htp = ctx.enter_context(tc.tile_pool(name="htp", bufs=17))
            w1p = ctx.enter_context(tc.tile_pool(name="w1p", bufs=12))
            w2p = ctx.enter_context(tc.tile_pool(name="w2p", bufs=6))
            gp = ctx.enter_context(tc.tile_pool(name="gp", bufs=4))
            up = ctx.enter_context(tc.tile_pool(name="up", bufs=4))
            ps = ctx.enter_context(tc.tile_pool(name="ps", bufs=8, space="PSUM"))

            # ---- per-channel epilogue constants (col m of [P, MT] holds
            # channels m*128..m*128+127: t[p, m] = aux[OFF + m*128 + p]) ----
            b1sb = const.tile([P, MT], f32)
            nc.gpsimd.dma_start(b1sb[:], bass.AP(aux, OFF_B1, [[1, P], [P, MT]]))
            b2sb = const.tile([P, MT], f32)
            nc.gpsimd.dma_start(b2sb[:], bass.AP(aux, OFF_B2, [[1, P], [P, MT]]))
            s1sb = const.tile([P, MT], f32)
            nc.gpsimd.dma_start(s1sb[:], bass.AP(aux, OFF_S1, [[1, P], [P, MT]]))
            s2sb = const.tile([P, MT], f32)
            nc.gpsimd.dma_start(s2sb[:], bass.AP(aux, OFF_S2, [[1, P], [P, MT]]))
            whf = const.tile([1, D], f32)
            nc.gpsimd.dma_start(whf[:], bass.AP(aux, OFF_WH, [[D, 1], [1, D]]))
            wh16 = const.tile([1, D], bf16)
            nc.vector.tensor_copy(wh16[:], whf[:])
            negC = const.tile([P, 1], f32)
            nc.vector.memset(negC[:], -45.0)

            def entropy_chain(hh):
                # windows straight from the host-supplied norms:
                #   wt[p, f, j] = m_ext[hh*1024 + p*16 + f + j]
                wt = smol.tile([64, 16, WIN], f32, name="wt", tag=f"wt{hh}")
                nc.gpsimd.dma_start(
                    wt[:], bass.AP(aux, OFF_M + hh * HALF,
                                   [[16, 64], [1, 16], [1, WIN]])
                )
                et = smol.tile([64, 16, WIN], f32, name="et", tag=f"et{hh}")
                nc.scalar.activation(et[:], wt[:], AF.Exp, bias=negC[:64])
                pw = smol.tile([64, 16, WIN], f32, name="pw", tag=f"pw{hh}")
                nc.vector.tensor_mul(pw[:], et[:], wt[:])
                S_ = smol.tile([64, 16], f32, name="S_", tag=f"S{hh}")
                nc.vector.reduce_sum(S_[:], et[:], axis=AX.X)
                T_ = smol.tile([64, 16], f32, name="T_", tag=f"T{hh}")
                nc.vector.reduce_sum(T_[:], pw[:], axis=AX.X)
                R_ = smol.tile([64, 16], f32, name="R_", tag=f"R{hh}")
                nc.vector.reciprocal(R_[:], S_[:])
                L_ = smol.tile([64, 16], f32, name="L_", tag=f"L{hh}")
                nc.scalar.activation(L_[:], S_[:], AF.Ln)
                U_ = smol.tile([64, 16], f32, name="U_", tag=f"U{hh}")
                nc.vector.tensor_mul(U_[:], T_[:], R_[:])
                V_ = smol.tile([64, 16], f32, name="V_", tag=f"V{hh}")
                nc.vector.tensor_sub(V_[:], L_[:], U_[:])
                Hb = smol.tile([64, 16], bf16, name="Hb", tag=f"Hb{hh}")
                nc.vector.tensor_scalar(
                    Hb[:], V_[:], 45.0, 1.4426950408889634,
                    op0=ALU.add, op1=ALU.mult,
                )
                nc.gpsimd.dma_start(bass.AP(h_scr[hh], 0, [[16, 64], [1, 16]]), Hb[:])

            # entropy for both halves depends only on the tiny aux DMA; run
            # it up front on ACT/DVE while the weight/activation streams load
            entropy_chain(0)
            entropy_chain(1)

            def load_gt(k, csl):
                qt = q8p.tile([P, HALF], i8, name="qt", tag="qt")
                nc.sync.dma_start(qt[:], q.ap()[k * P:(k + 1) * P, csl])
                gt = gate.tile([P, HALF], bf16, name="gt", tag="gt")
                nc.vector.tensor_copy(gt[:], qt[:])
                return gt

            def load_w1(k, mg):
                w8 = w18p.tile([P, 4 * P], i8, name="w8", tag="w8")
                nc.sync.dma_start(
                    w8[:], wq1.ap()[k * P:(k + 1) * P, mg * 512:(mg + 1) * 512]
                )
                wt_ = w1p.tile([P, 4 * P], bf16, name="wtile", tag="w1t")
                nc.vector.tensor_copy(wt_[:], w8[:])
                return wt_

            # ---- prologue: half-0 activation chunks + first-mg W1 chunks ----
            gts_half0 = []
            w1pre = []
            for k in range(KC):
                gts_half0.append(load_gt(k, slice(0, HALF)))
                if k < 10:
                    w1pre.append(load_w1(k, 0))

            # ---- main: two token-halves ----
            gts_by_half = {0: gts_half0}
            for h in range(2):
                gts = gts_by_half[h]
                hrow = const.tile([1, HALF], bf16, name="hrow", tag=f"hrow{h}")
                nc.gpsimd.dma_start(
                    hrow[:], bass.AP(h_scr[h], 0, [[HALF, 1], [1, HALF]])
                )

                hts = [htp.tile([P, HALF], bf16, name="ht", tag="ht")
                       for _ in range(MT)]

                # mm1: hT[m, tok] = silu(s1[m] * (sum_k W1q[k,m].T @ qT[k,tok]
                #                   + wh[m] * H[tok]) + b1[m])
                gts_next = []
                for mg in range(4):
                    pts = [[ps.tile([P, NT], f32, name="pt1", tag="pt")
                            for _ in range(2)] for _ in range(4)]
                    for k in range(KC):
                        if h == 0 and mg == 0 and k < len(w1pre):
                            wtile = w1pre[k]
                        else:
                            wtile = load_w1(k, mg)
                        for mi in range(4):
                            for n in range(2):
                                nc.tensor.matmul(
                                    pts[mi][n][:],
                                    wtile[:, mi * P:(mi + 1) * P],
                                    gts[k][:, n * NT:(n + 1) * NT],
                                    start=(k == 0), stop=False,
                                )
                        if h == 0 and mg == 3:
                            gts_next.append(load_gt(k, slice(HALF, 2 * HALF)))

                    for mi in range(4):
                        m = mg * 4 + mi
                        for n in range(2):
                            nc.tensor.matmul(
                                pts[mi][n][:],
                                wh16[:, m * P:(m + 1) * P],
                                hrow[:, n * NT:(n + 1) * NT],
                                start=False, stop=True,
                            )
                            nc.scalar.activation(
                                hts[m][:, n * NT:(n + 1) * NT], pts[mi][n][:],
                                AF.Silu, bias=b1sb[:, m:m + 1],
                                scale=s1sb[:, m:m + 1],
                            )

                if h == 0:
                    gts_by_half[1] = gts_next

                # mm2 + sigmoid -> quantized gate (small trailing groups cut
                # the tail; last group's W2 tiles prefetched early)
                w2pre = []
                for k2 in range(K2):
                    w8 = w28p.tile([P, 2 * P], i8, name="w28", tag="w28")
                    nc.sync.dma_start(
                        w8[:], wq2.ap()[k2 * P:(k2 + 1) * P, 14 * P:16 * P]
                    )
                    wpre = w2p.tile([P, 2 * P], bf16, name="w2pre", tag="w2s",
                                    bufs=17)
                    nc.vector.tensor_copy(wpre[:], w8[:])
                    w2pre.append(wpre)
                e_groups = [[0, 1, 2, 3], [4, 5, 6, 7], [8, 9, 10, 11],
                            [12, 13], [14, 15]]
                for egrp in e_groups:
                    ng = len(egrp)
                    pts2 = [[ps.tile([P, NT], f32, name="pt2", tag="pt")
                             for _ in range(2)] for _ in range(ng)]
                    for k2 in range(K2):
                        if egrp[0] == 14:
                            wtile2 = w2pre[k2]
                        else:
                            w8 = w28p.tile([P, ng * P], i8, name="w28", tag="w28")
                            nc.sync.dma_start(
                                w8[:], wq2.ap()[k2 * P:(k2 + 1) * P,
                                                egrp[0] * P:(egrp[-1] + 1) * P]
                            )
                            wtile2 = w2p.tile([P, ng * P], bf16, name="wtile2",
                                              tag="w2t")
                            nc.vector.tensor_copy(wtile2[:], w8[:])
                        for ei in range(ng):
                            for n in range(2):
                                nc.tensor.matmul(
                                    pts2[ei][n][:],
                                    wtile2[:, ei * P:(ei + 1) * P],
                                    hts[k2][:, n * NT:(n + 1) * NT],
                                    start=(k2 == 0), stop=(k2 == K2 - 1),
                                )
                    for ei in range(ng):
                        e = egrp[ei]
                        for n in range(2):
                            g = gp.tile([P, NT], f32, name="g", tag="g")
                            nc.scalar.activation(
                                g[:], pts2[ei][n][:], AF.Sigmoid,
                                bias=b2sb[:, e:e + 1], scale=s2sb[:, e:e + 1],
                            )
                            # DVE output conversion f32->u8 rounds to nearest
                            gu = up.tile([P, NT], u8, name="gu", tag="gu")
                            nc.vector.tensor_scalar_mul(gu[:], g[:], GSCALE)
                            nc.sync.dma_start(
                                gout.ap()[e * P:(e + 1) * P,
                                          h * HALF + n * NT:h * HALF + (n + 1) * NT],
                                gu[:],
                            )
    nc.finalize()
    return nc


def _get_ctx():
    """Build (once) the bass program and the jitted shard_map executor."""
    if "ctx" in _CACHE:
        return _CACHE["ctx"]
    import jax
    import jax.numpy as jnp
    import concourse.mybir as mybir
    from jax.sharding import Mesh, PartitionSpec, NamedSharding
    from jax.experimental.shard_map import shard_map
    from concourse.bass2jax import (
        _bass_exec_p, install_neuronx_cc_hook, partition_id_tensor,
    )

    nc = _build_nc()
    install_neuronx_cc_hook()
    partition_name = nc.partition_id_tensor.name if nc.partition_id_tensor else None
    in_names, out_names, out_avals = [], [], []
    for alloc in nc.m.functions[0].allocations:
        if not isinstance(alloc, mybir.MemoryLocationSet):
            continue
        name = alloc.memorylocations[0].name
        if alloc.kind == "ExternalInput":
            if name != partition_name:
                in_names.append(name)
        elif alloc.kind == "ExternalOutput":
            out_names.append(name)
            shape = tuple(alloc.tensor_shape)
            dtype = mybir.dt.np(alloc.dtype)
            out_avals.append(jax.core.ShapedArray(shape, dtype))
    n_params = len(in_names)
    n_outs = len(out_avals)
    all_names = list(in_names) + list(out_names)
    if partition_name is not None:
        all_names.append(partition_name)
    donate = tuple(range(n_params, n_params + n_outs))

    def _body(*args):
        operands = list(args)
        if partition_name is not None:
            operands.append(partition_id_tensor())
        outs = _bass_exec_p.bind(
            *operands,
            out_avals=tuple(out_avals),
            in_names=tuple(all_names),
            out_names=tuple(out_names),
            lowering_input_output_aliases=(),
            sim_require_finite=True,
            sim_require_nnan=True,
            nc=nc,
        )
        return tuple(outs)

    devices = jax.devices()[:N_CORES]
    mesh = Mesh(np.asarray(devices), ("core",))
    spec = PartitionSpec("core")
    sharded = jax.jit(
        shard_map(_body, mesh=mesh,
                  in_specs=(spec,) * (n_params + n_outs),
                  out_specs=(spec,) * n_outs,
                  check_rep=False),
        donate_argnums=donate, keep_unused=True,
    )
    shard = NamedSharding(mesh, spec)
    zero_fns = []
    for av in out_avals:
        gshape = (N_CORES * av.shape[0],) + av.shape[1:]

        def _mk(sh=gshape, dt=av.dtype):
            return jnp.zeros(sh, dt)

        zero_fns.append(jax.jit(_mk, out_shardings=shard))

    # per-device execution path: one jit, cached per input placement; each
    # core launches as soon as ITS operands are ready, so early cores' gate
    # fetches overlap later cores' input streaming (full-duplex tunnel)
    jitted = jax.jit(_body, donate_argnums=donate, keep_unused=True)
    dev_zero_fns = []
    for c in range(N_CORES):
        per_av = []
        for av in out_avals:
            sds = jax.sharding.SingleDeviceSharding(devices[c])

            def _mkd(sh=av.shape, dt=av.dtype):
                return jnp.zeros(sh, dt)

            per_av.append(jax.jit(_mkd, out_shardings=sds))
        dev_zero_fns.append(per_av)

    ctx = dict(nc=nc, sharded=sharded, in_names=in_names, out_names=out_names,
               out_avals=out_avals, mesh=mesh, devices=devices, shard=shard,
               zero_fns=zero_fns, jitted=jitted, dev_zero_fns=dev_zero_fns)
    _CACHE["ctx"] = ctx
    return ctx


def _make_in_maps(y_ssm, y_attn, x, W1, b1, W2, b2):
    """Host-side prep: transpose+quantize activations (per-feature scales
    folded into W1), per-column-quantized weights, token norms."""
    ys = np.asarray(y_ssm, np.float32).reshape(-1, D)
    ya = np.asarray(y_attn, np.float32).reshape(-1, D)
    xs = np.asarray(x, np.float32).reshape(-1, D)
    W1f = np.asarray(W1, np.float32)
    W2f = np.asarray(W2, np.float32)
    b1f = np.asarray(b1, np.float32)
    b2f = np.asarray(b2, np.float32)

    # per-feature activation scales (feature k = row k of the stacked qT)
    s_ys = np.maximum(np.abs(ys).max(axis=0), 1e-20)   # [D]
    s_ya = np.maximum(np.abs(ya).max(axis=0), 1e-20)
    qs = np.rint(ys.T * (127.0 / s_ys)[:, None]).astype(np.int8)  # [D, 16384]
    qa = np.rint(ya.T * (127.0 / s_ya)[:, None]).astype(np.int8)

    # fold activation dequant into W1, then per-output-column int8 quant
    s_feat = np.concatenate([s_ys, s_ya]) / 127.0      # [2D]
    A = W1f[:2 * D] * s_feat[:, None]                  # [2D, D]
    c1 = np.maximum(np.abs(A).max(axis=0), 1e-20)      # [D]
    qw1 = np.rint(A * (127.0 / c1)[None, :]).astype(np.int8)
    scale1 = (c1 / 127.0).astype(np.float32)
    w1h = (W1f[2 * D] / scale1).astype(np.float32)     # H row, pre-divided

    c2 = np.maximum(np.abs(W2f).max(axis=0), 1e-20)
    qw2 = np.rint(W2f * (127.0 / c2)[None, :]).astype(np.int8)
    scale2 = (c2 / 127.0).astype(np.float32)

    m = np.sqrt(np.einsum("td,td->t", xs, xs, optimize=True))  # [16384]

    aux_tail = np.empty(AUXN - OFF_S1, np.float32)
    aux_tail[OFF_S1 - OFF_S1:OFF_S2 - OFF_S1] = scale1
    aux_tail[OFF_S2 - OFF_S1:OFF_B1 - OFF_S1] = scale2
    aux_tail[OFF_B1 - OFF_S1:OFF_B2 - OFF_S1] = b1f
    aux_tail[OFF_B2 - OFF_S1:OFF_WH - OFF_S1] = b2f
    aux_tail[OFF_WH - OFF_S1:] = w1h

    in_maps = []
    for c in range(N_CORES):
        t0 = c * TOK
        qc = np.empty((2 * D, TOK), np.int8)
        qc[:D] = qs[:, t0:t0 + TOK]
        qc[D:] = qa[:, t0:t0 + TOK]
        auxc = np.zeros((AUXN,), np.float32)
        if t0 % S != 0:
            auxc[:WIN - 1] = m[t0 - (WIN - 1):t0]
        auxc[WIN - 1:EXT] = m[t0:t0 + TOK]
        auxc[OFF_S1:] = aux_tail
        in_maps.append({
            "q": qc,
            "aux": auxc,
            "wq1": qw1,
            "wq2": qw2,
        })
    return in_maps


def _run(in_maps, trace=False):
    """Place inputs (weights cross the wire once, then fan out D2D), launch
    each core's kernel as soon as its inputs are issued, and fetch each
    core's quantized gate in a background thread so fetches overlap later
    cores' input streaming (the tunnel is full-duplex). Returns list of
    per-core uint8 [D, TOK] arrays. All wire activity happens inside this
    call."""
    import jax
    from concurrent.futures import ThreadPoolExecutor

    ctx = _get_ctx()
    devices = ctx["devices"]
    gidx = ctx["out_names"].index("gout")

    # weights: one wire transfer to dev0 (two parallel streams, blocked on
    # arrival so they are not starved by the q streams below — every core's
    # launch depends on them), then device-to-device tree fanout (runs
    # terminal-side, overlapped with the activation puts below)
    shared_names = ["wq1", "wq2"]
    shared_dev = {}
    with ThreadPoolExecutor(max_workers=2) as wpool:
        wfuts = {
            name: wpool.submit(jax.device_put, in_maps[0][name], devices[0])
            for name in shared_names
        }
        for name in shared_names:
            buf = wfuts[name].result()
            buf.block_until_ready()
            shared_dev[name] = [buf]
    for step in range(3):                      # tree: 1 -> 2 -> 4 -> 8
        width = 1 << step
        for name in shared_names:
            bufs = shared_dev[name]
            for src in range(width):
                bufs.append(jax.device_put(bufs[src], devices[width + src]))

    try:
        # the tunnel throttles per stream (~44MB/s single, ~63MB/s with 4
        # streams): run 4 put streams of 2 staggered cores each, so early
        # cores' launches and gate fetches still pipeline under later
        # cores' input streaming
        with ThreadPoolExecutor(max_workers=4) as putpool, \
                ThreadPoolExecutor(max_workers=3) as fetchpool:

            def core_work(c):
                percore = {
                    name: jax.device_put(in_maps[c][name], devices[c])
                    for name in ["q", "aux"]
                }
                args = []
                for name in ctx["in_names"]:
                    args.append(percore[name] if name in percore
                                else shared_dev[name][c])
                for zf in ctx["dev_zero_fns"][c]:
                    args.append(zf())
                outs = ctx["jitted"](*args)
                return fetchpool.submit(np.asarray, outs[gidx])

            def group_work(cores):
                return [(c, core_work(c)) for c in cores]

            groups = [[0, 1], [2, 3], [4, 5], [6, 7]]
            gfuts = [putpool.submit(group_work, g) for g in groups]
            results = {}
            for gf in gfuts:
                for c, ff in gf.result():
                    results[c] = ff.result()
            return [results[c] for c in range(N_CORES)]
    except Exception:
        # fall back to the single shard_map launch (same program/math)
        return _run_shardmap(in_maps, ctx, shared_dev)


def _run_shardmap(in_maps, ctx, shared_dev=None):
    import jax

    devices = ctx["devices"]
    shard = ctx["shard"]
    if shared_dev is None:
        shared_dev = {}
        for name in ["wq1", "wq2"]:
            shared_dev[name] = [jax.device_put(in_maps[0][name], devices[0])]
        for step in range(3):
            width = 1 << step
            for name in ["wq1", "wq2"]:
                bufs = shared_dev[name]
                for src in range(width):
                    bufs.append(jax.device_put(bufs[src], devices[width + src]))
    percore_dev = {
        name: [jax.device_put(in_maps[c][name], devices[c])
               for c in range(N_CORES)]
        for name in ["q", "aux"]
    }

    def to_global(bufs):
        arr0 = bufs[0]
        gshape = (N_CORES * arr0.shape[0],) + tuple(arr0.shape[1:])
        return jax.make_array_from_single_device_arrays(gshape, shard, bufs)

    args = []
    for name in ctx["in_names"]:
        bufs = percore_dev[name] if name in percore_dev else shared_dev[name]
        args.append(to_global(bufs))
    for zf in ctx["zero_fns"]:
        args.append(zf())
    outs = ctx["sharded"](*args)
    gq_glob = outs[ctx["out_names"].index("gout")]
    shards = sorted(gq_glob.addressable_shards,
                    key=lambda s: s.index[0].start or 0)
    return [np.asarray(s.data) for s in shards]


def _mix(gq_shards, y_ssm, y_attn):
    """out = ya + g*(ys - ya) with g = gq/GSCALE, in f32 on host."""
    ys = np.asarray(y_ssm, np.float32).reshape(-1, D)
    ya = np.asarray(y_attn, np.float32).reshape(-1, D)
    out = np.empty_like(ys)
    for c in range(N_CORES):
        sl = slice(c * TOK, (c + 1) * TOK)
        g = gq_shards[c].T.astype(np.float32)
        g *= 1.0 / GSCALE
        out[sl] = ya[sl] + g * (ys[sl] - ya[sl])
    return out.reshape(B, S, D)


def kernel(y_ssm, y_attn, x, W1, b1, W2, b2):
    in_maps = _make_in_maps(y_ssm, y_attn, x, W1, b1, W2, b2)
    gq_shards = _run(in_maps)
    return _mix(gq_shards, y_ssm, y_attn).astype(np.float32)


# revision 14
# speedup vs baseline: 1.1170x; 1.1170x over previous
"""EntropyGate fused kernel for 8 Trainium2 NeuronCores (axon-tunneled).

Problem (hardcoded shapes): B=4, S=4096, D=2048, window=8.
  H = entropy of softmax over sliding causal window (8) of token L2 norms of x
  gate_in = [y_ssm | y_attn | H]  (B,S,2D+1)
  h = silu(gate_in @ W1 + b1); g = sigmoid(h @ W2 + b2)
  out = g*y_ssm + (1-g)*y_attn

Sharding: flatten tokens (B*S = 16384) -> 8 shards of 2048 tokens (each shard
lies within one sequence). Gate MLP weights replicated on-device via a
device-to-device broadcast (the axon host link is ~60MB/s; D2D is ~4x faster
and runs off the host wire).

Wire-traffic design (the axon tunnel dominates wall time; on-device compute
is ~1ms/core):
  - y_ssm/y_attn ship as int8 [2D, TOK] with per-feature scales folded into
    W1 host-side.
  - W1/W2 ship as int8 with per-output-column scales; the dequant scale is
    applied by the silu/sigmoid epilogue (activation computes
    func(in*scale + bias) and the psum partition dim IS the output channel).
    Combined quantization error lands ~5e-3 on the output, well inside the
    2e-2 gate.
  - token norms m = ||x_t|| ship as a tiny f32 vector per core instead of x
    itself (67MB); the windowed softmax entropy math stays on-device.
  - weights cross the wire once (to core 0) and fan out device-to-device.
  - the kernel returns the gate g quantized to uint8 (DVE converts f32->u8
    with round-to-nearest); the final elementwise mix out = ya + g*(ys-ya)
    runs on host in f32 from the original inputs.
  - donated output zero-buffers are created on-device; output shards are
    fetched in core order so early gates stream back while later cores'
    inputs are still going out (the tunnel is full-duplex).
"""

import numpy as np

P = 128
D = 2048
TOK = 2048        # tokens per core
HALF = 1024       # token half processed per pass
NT = 512          # psum n-tile (fp32 PSUM bank limit)
MT = 16           # d_out tiles of 128
KC = 32           # 128-row feature chunks of [qs; qa]
K2 = 16           # contraction chunks for mm2
WIN = 8
EXT = TOK + WIN - 1   # 2055
N_CORES = 8
B, S = 4, 4096
GSCALE = 255.0        # g is quantized as rne(g*GSCALE) into uint8

# aux tensor layout (f32), per core
OFF_M = 0             # m_ext: EXT norms (7-halo + TOK), zero-padded to 2176
OFF_S1 = 2176         # scale1[m] for mm1 epilogue (2048)
OFF_S2 = 4224         # scale2[e] for mm2 epilogue (2048)
OFF_B1 = 6272         # b1 (2048)
OFF_B2 = 8320         # b2 (2048)
OFF_WH = 10368        # W1 H-row / scale1 (2048)
AUXN = 12416

_CACHE = {}


def _build_nc():
    import concourse.bass as bass
    import concourse.tile as tile
    import concourse.mybir as mybir
    from concourse import bacc
    from contextlib import ExitStack

    f32 = mybir.dt.float32
    bf16 = mybir.dt.bfloat16
    i8 = mybir.dt.int8
    u8 = mybir.dt.uint8
    AF = mybir.ActivationFunctionType
    AX = mybir.AxisListType
    ALU = mybir.AluOpType

    nc = bacc.Bacc("TRN2", target_bir_lowering=False, debug=False, num_devices=1)

    q = nc.dram_tensor("q", [2 * D, TOK], i8, kind="ExternalInput")
    aux = nc.dram_tensor("aux", [AUXN], f32, kind="ExternalInput")
    wq1 = nc.dram_tensor("wq1", [2 * D, D], i8, kind="ExternalInput")
    wq2 = nc.dram_tensor("wq2", [D, D], i8, kind="ExternalInput")
    gout = nc.dram_tensor("gout", [D, TOK], u8, kind="ExternalOutput")
    # per-token-half entropy scratch (separate tensors keep the two entropy
    # pipelines independent in the dependency tracker)
    h_scr = [nc.dram_tensor(f"h_scr{i}", [HALF], bf16, kind="Internal")
             for i in range(2)]

    with tile.TileContext(nc) as tc:
        with ExitStack() as ctx:
            smol = ctx.enter_context(tc.tile_pool(name="smol", bufs=2))
            const = ctx.enter_context(tc.tile_pool(name="const", bufs=1))
            gate = ctx.enter_context(tc.tile_pool(name="gate", bufs=34))
            q8p = ctx.enter_context(tc.tile_pool(name="q8p", bufs=4))
            w18p = ctx.enter_context(tc.tile_pool(name="w18p", bufs=4))
            w28p = ctx.enter_context(tc.tile_pool(name="w28p", bufs=4))
            htp = ctx.enter_context(tc.tile_pool(name="htp", bufs=17))
            w1p = ctx.enter_context(tc.tile_pool(name="w1p", bufs=12))
            w2p = ctx.enter_context(tc.tile_pool(name="w2p", bufs=6))
            gp = ctx.enter_context(tc.tile_pool(name="gp", bufs=4))
            up = ctx.enter_context(tc.tile_pool(name="up", bufs=4))
            ps = ctx.enter_context(tc.tile_pool(name="ps", bufs=8, space="PSUM"))

            # ---- per-channel epilogue constants (col m of [P, MT] holds
            # channels m*128..m*128+127: t[p, m] = aux[OFF + m*128 + p]) ----
            b1sb = const.tile([P, MT], f32)
            nc.gpsimd.dma_start(b1sb[:], bass.AP(aux, OFF_B1, [[1, P], [P, MT]]))
            b2sb = const.tile([P, MT], f32)
            nc.gpsimd.dma_start(b2sb[:], bass.AP(aux, OFF_B2, [[1, P], [P, MT]]))
            s1sb = const.tile([P, MT], f32)
            nc.gpsimd.dma_start(s1sb[:], bass.AP(aux, OFF_S1, [[1, P], [P, MT]]))
            s2sb = const.tile([P, MT], f32)
            nc.gpsimd.dma_start(s2sb[:], bass.AP(aux, OFF_S2, [[1, P], [P, MT]]))
            whf = const.tile([1, D], f32)
            nc.gpsimd.dma_start(whf[:], bass.AP(aux, OFF_WH, [[D, 1], [1, D]]))
            wh16 = const.tile([1, D], bf16)
            nc.vector.tensor_copy(wh16[:], whf[:])
            negC = const.tile([P, 1], f32)
            nc.vector.memset(negC[:], -45.0)

            def entropy_chain(hh):
                # windows straight from the host-supplied norms:
                #   wt[p, f, j] = m_ext[hh*1024 + p*16 + f + j]
                wt = smol.tile([64, 16, WIN], f32, name="wt", tag=f"wt{hh}")
                nc.gpsimd.dma_start(
                    wt[:], bass.AP(aux, OFF_M + hh * HALF,
                                   [[16, 64], [1, 16], [1, WIN]])
                )
                et = smol.tile([64, 16, WIN], f32, name="et", tag=f"et{hh}")
                nc.scalar.activation(et[:], wt[:], AF.Exp, bias=negC[:64])
                pw = smol.tile([64, 16, WIN], f32, name="pw", tag=f"pw{hh}")
                nc.vector.tensor_mul(pw[:], et[:], wt[:])
                S_ = smol.tile([64, 16], f32, name="S_", tag=f"S{hh}")
                nc.vector.reduce_sum(S_[:], et[:], axis=AX.X)
                T_ = smol.tile([64, 16], f32, name="T_", tag=f"T{hh}")
                nc.vector.reduce_sum(T_[:], pw[:], axis=AX.X)
                R_ = smol.tile([64, 16], f32, name="R_", tag=f"R{hh}")
                nc.vector.reciprocal(R_[:], S_[:])
                L_ = smol.tile([64, 16], f32, name="L_", tag=f"L{hh}")
                nc.scalar.activation(L_[:], S_[:], AF.Ln)
                U_ = smol.tile([64, 16], f32, name="U_", tag=f"U{hh}")
                nc.vector.tensor_mul(U_[:], T_[:], R_[:])
                V_ = smol.tile([64, 16], f32, name="V_", tag=f"V{hh}")
                nc.vector.tensor_sub(V_[:], L_[:], U_[:])
                Hb = smol.tile([64, 16], bf16, name="Hb", tag=f"Hb{hh}")
                nc.vector.tensor_scalar(
                    Hb[:], V_[:], 45.0, 1.4426950408889634,
                    op0=ALU.add, op1=ALU.mult,
                )
                nc.gpsimd.dma_start(bass.AP(h_scr[hh], 0, [[16, 64], [1, 16]]), Hb[:])

            # entropy for both halves depends only on the tiny aux DMA; run
            # it up front on ACT/DVE while the weight/activation streams load
            entropy_chain(0)
            entropy_chain(1)

            def load_gt(k, csl):
                qt = q8p.tile([P, HALF], i8, name="qt", tag="qt")
                nc.sync.dma_start(qt[:], q.ap()[k * P:(k + 1) * P, csl])
                gt = gate.tile([P, HALF], bf16, name="gt", tag="gt")
                nc.vector.tensor_copy(gt[:], qt[:])
                return gt

            def load_w1(k, mg):
                w8 = w18p.tile([P, 4 * P], i8, name="w8", tag="w8")
                nc.sync.dma_start(
                    w8[:], wq1.ap()[k * P:(k + 1) * P, mg * 512:(mg + 1) * 512]
                )
                wt_ = w1p.tile([P, 4 * P], bf16, name="wtile", tag="w1t")
                nc.vector.tensor_copy(wt_[:], w8[:])
                return wt_

            # ---- prologue: half-0 activation chunks + first-mg W1 chunks ----
            gts_half0 = []
            w1pre = []
            for k in range(KC):
                gts_half0.append(load_gt(k, slice(0, HALF)))
                if k < 10:
                    w1pre.append(load_w1(k, 0))

            # ---- main: two token-halves ----
            gts_by_half = {0: gts_half0}
            for h in range(2):
                gts = gts_by_half[h]
                hrow = const.tile([1, HALF], bf16, name="hrow", tag=f"hrow{h}")
                nc.gpsimd.dma_start(
                    hrow[:], bass.AP(h_scr[h], 0, [[HALF, 1], [1, HALF]])
                )

                hts = [htp.tile([P, HALF], bf16, name="ht", tag="ht")
                       for _ in range(MT)]

                # mm1: hT[m, tok] = silu(s1[m] * (sum_k W1q[k,m].T @ qT[k,tok]
                #                   + wh[m] * H[tok]) + b1[m])
                gts_next = []
                for mg in range(4):
                    pts = [[ps.tile([P, NT], f32, name="pt1", tag="pt")
                            for _ in range(2)] for _ in range(4)]
                    for k in range(KC):
                        if h == 0 and mg == 0 and k < len(w1pre):
                            wtile = w1pre[k]
                        else:
                            wtile = load_w1(k, mg)
                        for mi in range(4):
                            for n in range(2):
                                nc.tensor.matmul(
                                    pts[mi][n][:],
                                    wtile[:, mi * P:(mi + 1) * P],
                                    gts[k][:, n * NT:(n + 1) * NT],
                                    start=(k == 0), stop=False,
                                )
                        if h == 0 and mg == 3:
                            gts_next.append(load_gt(k, slice(HALF, 2 * HALF)))

                    for mi in range(4):
                        m = mg * 4 + mi
                        for n in range(2):
                            nc.tensor.matmul(
                                pts[mi][n][:],
                                wh16[:, m * P:(m + 1) * P],
                                hrow[:, n * NT:(n + 1) * NT],
                                start=False, stop=True,
                            )
                            nc.scalar.activation(
                                hts[m][:, n * NT:(n + 1) * NT], pts[mi][n][:],
                                AF.Silu, bias=b1sb[:, m:m + 1],
                                scale=s1sb[:, m:m + 1],
                            )

                if h == 0:
                    gts_by_half[1] = gts_next

                # mm2 + sigmoid -> quantized gate (small trailing groups cut
                # the tail; last group's W2 tiles prefetched early)
                w2pre = []
                for k2 in range(K2):
                    w8 = w28p.tile([P, 2 * P], i8, name="w28", tag="w28")
                    nc.sync.dma_start(
                        w8[:], wq2.ap()[k2 * P:(k2 + 1) * P, 14 * P:16 * P]
                    )
                    wpre = w2p.tile([P, 2 * P], bf16, name="w2pre", tag="w2s",
                                    bufs=17)
                    nc.vector.tensor_copy(wpre[:], w8[:])
                    w2pre.append(wpre)
                e_groups = [[0, 1, 2, 3], [4, 5, 6, 7], [8, 9, 10, 11],
                            [12, 13], [14, 15]]
                for egrp in e_groups:
                    ng = len(egrp)
                    pts2 = [[ps.tile([P, NT], f32, name="pt2", tag="pt")
                             for _ in range(2)] for _ in range(ng)]
                    for k2 in range(K2):
                        if egrp[0] == 14:
                            wtile2 = w2pre[k2]
                        else:
                            w8 = w28p.tile([P, ng * P], i8, name="w28", tag="w28")
                            nc.sync.dma_start(
                                w8[:], wq2.ap()[k2 * P:(k2 + 1) * P,
                                                egrp[0] * P:(egrp[-1] + 1) * P]
                            )
                            wtile2 = w2p.tile([P, ng * P], bf16, name="wtile2",
                                              tag="w2t")
                            nc.vector.tensor_copy(wtile2[:], w8[:])
                        for ei in range(ng):
                            for n in range(2):
                                nc.tensor.matmul(
                                    pts2[ei][n][:],
                                    wtile2[:, ei * P:(ei + 1) * P],
                                    hts[k2][:, n * NT:(n + 1) * NT],
                                    start=(k2 == 0), stop=(k2 == K2 - 1),
                                )
                    for ei in range(ng):
                        e = egrp[ei]
                        for n in range(2):
                            g = gp.tile([P, NT], f32, name="g", tag="g")
                            nc.scalar.activation(
                                g[:], pts2[ei][n][:], AF.Sigmoid,
                                bias=b2sb[:, e:e + 1], scale=s2sb[:, e:e + 1],
                            )
                            # DVE output conversion f32->u8 rounds to nearest
                            gu = up.tile([P, NT], u8, name="gu", tag="gu")
                            nc.vector.tensor_scalar_mul(gu[:], g[:], GSCALE)
                            nc.sync.dma_start(
                                gout.ap()[e * P:(e + 1) * P,
                                          h * HALF + n * NT:h * HALF + (n + 1) * NT],
                                gu[:],
                            )
    nc.finalize()
    return nc


def _get_ctx():
    """Build (once) the bass program and the jitted shard_map executor."""
    if "ctx" in _CACHE:
        return _CACHE["ctx"]
    import jax
    import jax.numpy as jnp
    import concourse.mybir as mybir
    from jax.sharding import Mesh, PartitionSpec, NamedSharding
    from jax.experimental.shard_map import shard_map
    from concourse.bass2jax import (
        _bass_exec_p, install_neuronx_cc_hook, partition_id_tensor,
    )

    nc = _build_nc()
    install_neuronx_cc_hook()
    partition_name = nc.partition_id_tensor.name if nc.partition_id_tensor else None
    in_names, out_names, out_avals = [], [], []
    for alloc in nc.m.functions[0].allocations:
        if not isinstance(alloc, mybir.MemoryLocationSet):
            continue
        name = alloc.memorylocations[0].name
        if alloc.kind == "ExternalInput":
            if name != partition_name:
                in_names.append(name)
        elif alloc.kind == "ExternalOutput":
            out_names.append(name)
            shape = tuple(alloc.tensor_shape)
            dtype = mybir.dt.np(alloc.dtype)
            out_avals.append(jax.core.ShapedArray(shape, dtype))
    n_params = len(in_names)
    n_outs = len(out_avals)
    all_names = list(in_names) + list(out_names)
    if partition_name is not None:
        all_names.append(partition_name)
    donate = tuple(range(n_params, n_params + n_outs))

    def _body(*args):
        operands = list(args)
        if partition_name is not None:
            operands.append(partition_id_tensor())
        outs = _bass_exec_p.bind(
            *operands,
            out_avals=tuple(out_avals),
            in_names=tuple(all_names),
            out_names=tuple(out_names),
            lowering_input_output_aliases=(),
            sim_require_finite=True,
            sim_require_nnan=True,
            nc=nc,
        )
        return tuple(outs)

    devices = jax.devices()[:N_CORES]
    mesh = Mesh(np.asarray(devices), ("core",))
    spec = PartitionSpec("core")
    sharded = jax.jit(
        shard_map(_body, mesh=mesh,
                  in_specs=(spec,) * (n_params + n_outs),
                  out_specs=(spec,) * n_outs,
                  check_rep=False),
        donate_argnums=donate, keep_unused=True,
    )
    shard = NamedSharding(mesh, spec)
    zero_fns = []
    for av in out_avals:
        gshape = (N_CORES * av.shape[0],) + av.shape[1:]

        def _mk(sh=gshape, dt=av.dtype):
            return jnp.zeros(sh, dt)

        zero_fns.append(jax.jit(_mk, out_shardings=shard))

    # per-device execution path: one jit, cached per input placement; each
    # core launches as soon as ITS operands are ready, so early cores' gate
    # fetches overlap later cores' input streaming (full-duplex tunnel)
    jitted = jax.jit(_body, donate_argnums=donate, keep_unused=True)
    dev_zero_fns = []
    for c in range(N_CORES):
        per_av = []
        for av in out_avals:
            sds = jax.sharding.SingleDeviceSharding(devices[c])

            def _mkd(sh=av.shape, dt=av.dtype):
                return jnp.zeros(sh, dt)

            per_av.append(jax.jit(_mkd, out_shardings=sds))
        dev_zero_fns.append(per_av)

    ctx = dict(nc=nc, sharded=sharded, in_names=in_names, out_names=out_names,
               out_avals=out_avals, mesh=mesh, devices=devices, shard=shard,
               zero_fns=zero_fns, jitted=jitted, dev_zero_fns=dev_zero_fns)
    _CACHE["ctx"] = ctx
    return ctx


def _make_in_maps(y_ssm, y_attn, x, W1, b1, W2, b2):
    """Host-side prep: transpose+quantize activations (per-feature scales
    folded into W1), per-column-quantized weights, token norms."""
    ys = np.asarray(y_ssm, np.float32).reshape(-1, D)
    ya = np.asarray(y_attn, np.float32).reshape(-1, D)
    xs = np.asarray(x, np.float32).reshape(-1, D)
    W1f = np.asarray(W1, np.float32)
    W2f = np.asarray(W2, np.float32)
    b1f = np.asarray(b1, np.float32)
    b2f = np.asarray(b2, np.float32)

    # per-feature activation scales (feature k = row k of the stacked qT)
    s_ys = np.maximum(np.abs(ys).max(axis=0), 1e-20)   # [D]
    s_ya = np.maximum(np.abs(ya).max(axis=0), 1e-20)
    qs = np.rint(ys.T * (127.0 / s_ys)[:, None]).astype(np.int8)  # [D, 16384]
    qa = np.rint(ya.T * (127.0 / s_ya)[:, None]).astype(np.int8)

    # fold activation dequant into W1, then per-output-column int8 quant
    s_feat = np.concatenate([s_ys, s_ya]) / 127.0      # [2D]
    A = W1f[:2 * D] * s_feat[:, None]                  # [2D, D]
    c1 = np.maximum(np.abs(A).max(axis=0), 1e-20)      # [D]
    qw1 = np.rint(A * (127.0 / c1)[None, :]).astype(np.int8)
    scale1 = (c1 / 127.0).astype(np.float32)
    w1h = (W1f[2 * D] / scale1).astype(np.float32)     # H row, pre-divided

    c2 = np.maximum(np.abs(W2f).max(axis=0), 1e-20)
    qw2 = np.rint(W2f * (127.0 / c2)[None, :]).astype(np.int8)
    scale2 = (c2 / 127.0).astype(np.float32)

    m = np.sqrt(np.einsum("td,td->t", xs, xs, optimize=True))  # [16384]

    aux_tail = np.empty(AUXN - OFF_S1, np.float32)
    aux_tail[OFF_S1 - OFF_S1:OFF_S2 - OFF_S1] = scale1
    aux_tail[OFF_S2 - OFF_S1:OFF_B1 - OFF_S1] = scale2
    aux_tail[OFF_B1 - OFF_S1:OFF_B2 - OFF_S1] = b1f
    aux_tail[OFF_B2 - OFF_S1:OFF_WH - OFF_S1] = b2f
    aux_tail[OFF_WH - OFF_S1:] = w1h

    in_maps = []
    for c in range(N_CORES):
        t0 = c * TOK
        qc = np.empty((2 * D, TOK), np.int8)
        qc[:D] = qs[:, t0:t0 + TOK]
        qc[D:] = qa[:, t0:t0 + TOK]
        auxc = np.zeros((AUXN,), np.float32)
        if t0 % S != 0:
            auxc[:WIN - 1] = m[t0 - (WIN - 1):t0]
        auxc[WIN - 1:EXT] = m[t0:t0 + TOK]
        auxc[OFF_S1:] = aux_tail
        in_maps.append({
            "q": qc,
            "aux": auxc,
            "wq1": qw1,
            "wq2": qw2,
        })
    return in_maps


def _run(in_maps, trace=False):
    """Place inputs (weights cross the wire once, then fan out D2D), launch
    each core's kernel as soon as its inputs are issued, and fetch each
    core's quantized gate in a background thread so fetches overlap later
    cores' input streaming (the tunnel is full-duplex). Returns list of
    per-core uint8 [D, TOK] arrays. All wire activity happens inside this
    call."""
    import jax
    from concurrent.futures import ThreadPoolExecutor

    ctx = _get_ctx()
    devices = ctx["devices"]
    gidx = ctx["out_names"].index("gout")

    # weights: one wire transfer to dev0 (first in the single put stream, so
    # they arrive before any core's activations), then device-to-device tree
    # fanout (runs terminal-side, overlapped with the activation puts below).
    # NOTE: parallel put streams pump the tunnel ~25% faster in isolation,
    # but bunch all cores' arrivals together so the per-core launch+fetch
    # pipeline collapses — measured slower and noisier end to end.
    shared_names = ["wq1", "wq2"]
    shared_dev = {}
    for name in shared_names:
        shared_dev[name] = [jax.device_put(in_maps[0][name], devices[0])]
    for step in range(3):                      # tree: 1 -> 2 -> 4 -> 8
        width = 1 << step
        for name in shared_names:
            bufs = shared_dev[name]
            for src in range(width):
                bufs.append(jax.device_put(bufs[src], devices[width + src]))

    try:
        with ThreadPoolExecutor(max_workers=2) as pool:
            futs = []
            for c in range(N_CORES):
                percore = {
                    name: jax.device_put(in_maps[c][name], devices[c])
                    for name in ["q", "aux"]
                }
                args = []
                for name in ctx["in_names"]:
                    args.append(percore[name] if name in percore
                                else shared_dev[name][c])
                for zf in ctx["dev_zero_fns"][c]:
                    args.append(zf())
                outs = ctx["jitted"](*args)
                futs.append(pool.submit(np.asarray, outs[gidx]))
            return [f.result() for f in futs]
    except Exception:
        # fall back to the single shard_map launch (same program/math)
        return _run_shardmap(in_maps, ctx, shared_dev)


def _run_shardmap(in_maps, ctx, shared_dev=None):
    import jax

    devices = ctx["devices"]
    shard = ctx["shard"]
    if shared_dev is None:
        shared_dev = {}
        for name in ["wq1", "wq2"]:
            shared_dev[name] = [jax.device_put(in_maps[0][name], devices[0])]
        for step in range(3):
            width = 1 << step
            for name in ["wq1", "wq2"]:
                bufs = shared_dev[name]
                for src in range(width):
                    bufs.append(jax.device_put(bufs[src], devices[width + src]))
    percore_dev = {
        name: [jax.device_put(in_maps[c][name], devices[c])
               for c in range(N_CORES)]
        for name in ["q", "aux"]
    }

    def to_global(bufs):
        arr0 = bufs[0]
        gshape = (N_CORES * arr0.shape[0],) + tuple(arr0.shape[1:])
        return jax.make_array_from_single_device_arrays(gshape, shard, bufs)

    args = []
    for name in ctx["in_names"]:
        bufs = percore_dev[name] if name in percore_dev else shared_dev[name]
        args.append(to_global(bufs))
    for zf in ctx["zero_fns"]:
        args.append(zf())
    outs = ctx["sharded"](*args)
    gq_glob = outs[ctx["out_names"].index("gout")]
    shards = sorted(gq_glob.addressable_shards,
                    key=lambda s: s.index[0].start or 0)
    return [np.asarray(s.data) for s in shards]


def _mix(gq_shards, y_ssm, y_attn):
    """out = ya + g*(ys - ya) with g = gq/GSCALE, in f32 on host."""
    ys = np.asarray(y_ssm, np.float32).reshape(-1, D)
    ya = np.asarray(y_attn, np.float32).reshape(-1, D)
    out = np.empty_like(ys)
    for c in range(N_CORES):
        sl = slice(c * TOK, (c + 1) * TOK)
        g = gq_shards[c].T.astype(np.float32)
        g *= 1.0 / GSCALE
        out[sl] = ya[sl] + g * (ys[sl] - ya[sl])
    return out.reshape(B, S, D)


def kernel(y_ssm, y_attn, x, W1, b1, W2, b2):
    in_maps = _make_in_maps(y_ssm, y_attn, x, W1, b1, W2, b2)
    gq_shards = _run(in_maps)
    return _mix(gq_shards, y_ssm, y_attn).astype(np.float32)
